# revision 27
# baseline (speedup 1.0000x reference)
"""GNN message-passing kernel for 8 Trainium2 NeuronCores (Bass/Tile).

Takes FULL inputs, shards nodes across 8 cores internally, runs the
4-layer GNN (dense -> spmm -> spmm -> dense) with bf16 AllGathers of the
hidden node table (split into X/Y sub-tables so the second collective
overlaps the first sub-table's gathers), and PE-matmul-based weighted
segment sums (host-built one-hot selector matrices), then gathers the
full output. dma_gather calls rotate over 4 SWDGE queues so descriptor
generation runs on all four Q7 core-pairs concurrently.
"""

import math
from contextlib import ExitStack
from dataclasses import dataclass

import ml_dtypes
import numpy as np

import concourse.bass as bass
import concourse.mybir as mybir
import concourse.tile as tile
from concourse import bacc
from concourse.bass_utils import run_bass_kernel_spmd
from concourse.masks import make_identity

BF16 = ml_dtypes.bfloat16
AF = mybir.ActivationFunctionType


@dataclass(frozen=True)
class Cfg:
    n_nodes: int = 50000
    n_edges: int = 800000
    in_dim: int = 512
    h1: int = 512
    h2: int = 256
    out_dim: int = 128
    n_cores: int = 8
    group_blocks: int = 3  # row-blocks per gather group
    sub_blocks: tuple = (12, 12, 12, 13)  # local row-blocks per sub-table

    @property
    def nodes_per_core(self):
        return math.ceil(self.n_nodes / self.n_cores)

    @property
    def npad(self):  # per-core padded nodes
        return math.ceil(self.nodes_per_core / 128) * 128

    @property
    def nblocks(self):
        return self.npad // 128

    @property
    def nsub(self):
        return len(self.sub_blocks)

    @property
    def sub_cumblk(self):  # cumulative block boundaries of the sub-tables
        c = [0]
        for nb_ in self.sub_blocks:
            c.append(c[-1] + nb_)
        return c

    @property
    def srows(self):  # per-core rows per sub-table
        return [nb_ * 128 for nb_ in self.sub_blocks]

    @property
    def stot(self):  # total rows per sub-table across cores
        return [r * self.n_cores for r in self.srows]

    @property
    def ngroups(self):
        return math.ceil(self.nblocks / self.group_blocks)


FULL = Cfg()

CALL_CHUNKS = 5  # chunks per dma_gather call (fits the SWDGE ring 3x over)


# ---------------------------------------------------------------- host prep


def edge_structure(cfg: Cfg, edge_row, edge_col, edge_weight):
    """Per-core edge streams with SPMD-uniform chunk counts.

    Edges are bucketed per (core, row-block, sub-table) where the
    sub-table (h=0: X, h=1: Y) is chosen by the column's local row so
    that each sub-table fits int16 gather indices.

    Returns (meta, per_core) where meta has the uniform chunk structure:
      meta['nch'][g][h]      total chunks in group g, sub-table h
      meta['chunk_blk'][g][h] list of block ids (one per chunk, ordered)
      meta['off16'][g][h]    idx-tile column offset (int16 cols)
      meta['totch']          total chunks
      meta['idxcols']        total int16 columns of the idx tensor
    per_core[c] = dict(idx=[128, idxcols] int16, pmat=[128, totch*128] bf16)
    """
    nc_, npad, nb, G = (cfg.n_cores, cfg.npad, cfg.nblocks, cfg.group_blocks)
    npc = cfg.nodes_per_core
    NS = cfg.nsub
    assert all(t <= 32767 for t in cfg.stot)
    bounds = np.array([b * 128 for b in cfg.sub_cumblk])  # local-row bounds
    srows = np.array(cfg.srows)

    core_of = edge_row // npc
    lr_all = edge_row - core_of * npc  # local row
    ccore = edge_col // npc
    clocal = edge_col - ccore * npc
    sub_all = np.searchsorted(bounds, clocal, side="right") - 1  # sub-table
    cl_all = ccore * srows[sub_all] + (clocal - bounds[sub_all])

    # bucket edges per (core, group, sub); blocks stay ordered inside a
    # group stream so a chunk spans at most a couple of adjacent blocks
    per = {}
    ngroups = cfg.ngroups
    counts = np.zeros((nc_, ngroups, NS), np.int64)
    for c in range(nc_):
        m = core_of == c
        lr, cl, hf, w = lr_all[m], cl_all[m], sub_all[m], edge_weight[m]
        blk = lr // 128
        grp = blk // G
        order = np.lexsort((lr, blk, hf, grp))
        per[c] = (lr[order], cl[order], hf[order], w[order], blk[order], grp[order])
        np.add.at(counts[c], (grp, hf), 1)

    # uniform chunks per (group, sub) = max over cores
    chunks_gh = np.ceil(counts / 128.0).astype(np.int64).max(axis=0)

    nch = [[int(chunks_gh[g, h]) for h in range(NS)] for g in range(ngroups)]
    off16 = [[0] * NS for _ in range(ngroups)]
    tot16 = 0
    for g in range(ngroups):
        for h in range(NS):
            off16[g][h] = tot16
            tot16 += nch[g][h] * 8  # 128 idx per chunk -> 8 int16 cols

    # per-core block layout inside each (g, h) stream, then the UNION of
    # (chunk, block) MM pairs across cores (a chunk near a per-core block
    # boundary may need MMs into two adjacent blocks)
    mm_sets = [[{} for _ in range(NS)] for _ in range(ngroups)]  # {(j, b)}
    core_spans = {}  # (c, g, h) -> list of (block, start_slot, end_slot)
    for c in range(nc_):
        lr, cl, hf, w, blk, grp = per[c]
        for g in range(ngroups):
            for h in range(NS):
                sel = (grp == g) & (hf == h)
                bsel = blk[sel]
                spans = []
                pos = 0
                for b in range(g * G, min((g + 1) * G, nb)):
                    nB = int((bsel == b).sum())
                    if nB:
                        spans.append((b, pos, pos + nB))
                        for j in range(pos // 128, (pos + nB - 1) // 128 + 1):
                            mm_sets[g][h].setdefault((j, b), True)
                    pos += nB
                core_spans[(c, g, h)] = spans

    mm_list = [
        [sorted(mm_sets[g][h].keys()) for h in range(NS)]
        for g in range(ngroups)
    ]
    totmm = sum(
        len(mm_list[g][h]) for g in range(ngroups) for h in range(NS)
    )
    totch = int(chunks_gh.sum())

    meta = dict(
        nch=nch,
        mm_list=mm_list,
        off16=off16,
        totch=totch,
        totmm=totmm,
        idxcols=max(tot16, 8),
    )

    # global pmat column index of each (g, h, j, b) MM
    mm_col = {}
    jj = 0
    for g in range(ngroups):
        for h in range(NS):
            for key in mm_list[g][h]:
                mm_col[(g, h) + key] = jj
                jj += 1

    per_core = []
    for c in range(nc_):
        lr, cl, hf, w, blk, grp = per[c]
        idx_flat = np.zeros(meta["idxcols"] * 16, np.int16)
        pmat = np.zeros((128, totmm * 128), BF16)
        for g in range(ngroups):
            for h in range(NS):
                sel = (grp == g) & (hf == h)
                e_cl, e_lr, e_w = cl[sel], lr[sel], w[sel]
                n = len(e_cl)
                if n == 0:
                    continue
                # idx stream (pad slots keep idx 0 -> harmless row-0 reads)
                base16 = off16[g][h]
                i_in = np.arange(n)
                idx_flat[(base16 + i_in // 16) * 16 + (i_in % 16)] = (
                    e_cl.astype(np.int16)
                )
                # P matrices per (chunk, block) MM
                for b, s0, s1 in core_spans[(c, g, h)]:
                    i_e = np.arange(s0, s1)
                    jloc = i_e // 128
                    slot = i_e % 128
                    col = np.array(
                        [mm_col[(g, h, int(j), b)] for j in jloc], np.int64
                    )
                    r = e_lr[s0:s1] - b * 128
                    pmat[slot, col * 128 + r] = e_w[s0:s1].astype(BF16)
        idx_mat = idx_flat.reshape(meta["idxcols"], 16).T  # [16, idxcols]
        idx_mat = np.tile(idx_mat, (8, 1))  # replicate to 128 partitions
        per_core.append(dict(idx=np.ascontiguousarray(idx_mat), pmat=pmat))

    return meta, per_core


def prep_inputs(cfg: Cfg, inputs):
    """Shard + lay out all per-core input tensors."""
    f = inputs["features"].astype(np.float32)
    meta, per_edge = edge_structure(
        cfg,
        inputs["edge_row"].astype(np.int64),
        inputs["edge_col"].astype(np.int64),
        inputs["edge_weight"].astype(np.float32),
    )
    kin = cfg.in_dim // 128
    k1 = cfg.h1 // 128
    k2 = cfg.h2 // 128

    def wlayout(w, kt):
        # [K, M] -> [128, kt*M] with [:, i*M:(i+1)*M] = w[i*128:(i+1)*128]
        K, M = w.shape
        return (
            w.reshape(kt, 128, M).transpose(1, 0, 2).reshape(128, kt * M)
        ).astype(BF16)

    w1 = wlayout(inputs["W_lin1"].astype(np.float32), kin)
    wg1 = wlayout(inputs["W_g1"].astype(np.float32), k1)
    wg2 = wlayout(inputs["W_g2"].astype(np.float32), k2)
    wl2 = wlayout(inputs["W_lin2"].astype(np.float32), k2)
    b1 = (
        inputs["b_lin1"].astype(np.float32).reshape(kin, 128).T.copy()
    )  # [128, kin]
    bg1 = inputs["b_g1"].astype(BF16).reshape(1, cfg.h2)
    bg2 = inputs["b_g2"].astype(BF16).reshape(1, cfg.h2)
    bl2 = inputs["b_lin2"].astype(BF16).reshape(1, cfg.out_dim)

    npc, npad = cfg.nodes_per_core, cfg.npad
    in_maps = []
    for c in range(cfg.n_cores):
        lo = c * npc
        hi = min((c + 1) * npc, cfg.n_nodes)
        xc = np.zeros((npad, cfg.in_dim), np.float32)
        xc[: hi - lo] = f[lo:hi]
        # XT layout [128, kin*npad]: [:, i*npad:(i+1)*npad] = x.T[i*128:...]
        xt = (
            xc.T.reshape(kin, 128, npad)
            .transpose(1, 0, 2)
            .reshape(128, kin * npad)
        ).astype(BF16)
        in_maps.append(
            {
                "xt": np.ascontiguousarray(xt),
                "w1": w1,
                "wg1": wg1,
                "wg2": wg2,
                "wl2": wl2,
                "b1": b1,
                "bg1": bg1,
                "bg2": bg2,
                "bl2": bl2,
                "idx": per_edge[c]["idx"],
                "pmat": per_edge[c]["pmat"],
            }
        )
    return meta, in_maps


# ---------------------------------------------------------------- kernel IR


def build(cfg: Cfg, meta):
    nc = bacc.Bacc(
        "TRN2",
        target_bir_lowering=False,
        debug=False,
        num_devices=cfg.n_cores,
        num_swdge_queues=4,
        dynamic_dma_scratch_size=32768,
    )
    bf = mybir.dt.bfloat16
    f32 = mybir.dt.float32
    i16 = mybir.dt.int16
    kin = cfg.in_dim // 128
    k1 = cfg.h1 // 128
    k2 = cfg.h2 // 128
    npad, nb, G, H2, OUT = (
        cfg.npad,
        cfg.nblocks,
        cfg.group_blocks,
        cfg.h2,
        cfg.out_dim,
    )
    NS = cfg.nsub
    CUM = cfg.sub_cumblk
    totch = meta["totch"]

    xt_d = nc.dram_tensor("xt", [128, kin * npad], bf, kind="ExternalInput").ap()
    w1_d = nc.dram_tensor("w1", [128, kin * cfg.h1], bf, kind="ExternalInput").ap()
    wg1_d = nc.dram_tensor("wg1", [128, k1 * H2], bf, kind="ExternalInput").ap()
    wg2_d = nc.dram_tensor("wg2", [128, k2 * H2], bf, kind="ExternalInput").ap()
    wl2_d = nc.dram_tensor("wl2", [128, k2 * OUT], bf, kind="ExternalInput").ap()
    b1_d = nc.dram_tensor("b1", [128, kin], f32, kind="ExternalInput").ap()
    bg1_d = nc.dram_tensor("bg1", [1, H2], bf, kind="ExternalInput").ap()
    bg2_d = nc.dram_tensor("bg2", [1, H2], bf, kind="ExternalInput").ap()
    bl2_d = nc.dram_tensor("bl2", [1, OUT], bf, kind="ExternalInput").ap()
    idx_d = nc.dram_tensor(
        "idx", [128, meta["idxcols"]], i16, kind="ExternalInput"
    ).ap()
    pmat_d = nc.dram_tensor(
        "pmat", [128, meta["totmm"] * 128], bf, kind="ExternalInput"
    ).ap()
    y_d = nc.dram_tensor("y", [npad, OUT], f32, kind="ExternalOutput").ap()

    g1l = [
        nc.dram_tensor(f"g1l{s}", [cfg.srows[s], H2], bf).ap()
        for s in range(NS)
    ]
    g2l = [
        nc.dram_tensor(f"g2l{s}", [cfg.srows[s], H2], bf).ap()
        for s in range(NS)
    ]
    g1t = [
        nc.dram_tensor(f"g1t{s}", [cfg.stot[s], H2], bf, addr_space="Shared").ap()
        for s in range(NS)
    ]
    g2t = [
        nc.dram_tensor(f"g2t{s}", [cfg.stot[s], H2], bf, addr_space="Shared").ap()
        for s in range(NS)
    ]

    rg = [list(range(cfg.n_cores))]

    def allgather(local, table):
        nc.gpsimd.collective_compute(
            "AllGather",
            mybir.AluOpType.bypass,
            replica_groups=rg,
            ins=[local[:, :]],
            outs=[table[:, :]],
        )

    def spmm(
        tc,
        ctx,
        nc,
        tables,
        idx_s,
        ones_t,
        brow,
        psum_tag,
        out_cb,
        pref,
        pre_hook=None,
    ):
        """Weighted segment-sum of gathered table rows, per row-block.

        Sub-table-0 gathers are emitted `pref` groups ahead of their
        consuming matmuls so the GpSimd queue never head-of-line blocks
        on the later sub-tables' AllGathers. `pre_hook` (e.g. the last
        sub-table's AllGather) is emitted after the prefetch gathers so
        they aren't queued behind it.
        """
        calls_per_gh = max(
            math.ceil(meta["nch"][g][h] / CALL_CHUNKS)
            for g in range(cfg.ngroups)
            for h in range(NS)
        )
        gp0 = ctx.enter_context(
            tc.tile_pool(name=f"g0{psum_tag}", bufs=(pref + 2) * calls_per_gh)
        )
        gps = [
            ctx.enter_context(
                tc.tile_pool(name=f"g{h}{psum_tag}", bufs=2 * calls_per_gh)
            )
            for h in range(1, NS)
        ]
        pp = ctx.enter_context(tc.tile_pool(name=f"pm{psum_tag}", bufs=2))
        sp = ctx.enter_context(
            tc.tile_pool(name=f"ps{psum_tag}", bufs=G + 1, space="PSUM")
        )
        qctr = [0]

        def emit_gathers(g, h, pool):
            n = meta["nch"][g][h]
            tiles = []
            for lo in range(0, n, CALL_CHUNKS):
                ns = min(CALL_CHUNKS, n - lo)
                t = pool.tile([128, CALL_CHUNKS, H2], bf, tag="g")
                o16 = meta["off16"][g][h] + lo * 8
                nc.gpsimd.dma_gather(
                    out_ap=t[:, :ns, :],
                    in_ap=tables[h][:, :],
                    idxs_ap=idx_s[:, o16 : o16 + ns * 8],
                    num_idxs=ns * 128,
                    num_idxs_reg=ns * 128,
                    elem_size=H2,
                    single_packet=False,
                    queue_num=qctr[0] % 4,
                )
                qctr[0] += 1
                tiles.append(t)
            return tiles

        xtiles = {}
        for gg in range(min(pref + 1, cfg.ngroups)):
            xtiles[gg] = emit_gathers(gg, 0, gp0)
        if pre_hook is not None:
            pre_hook()

        j0 = 0
        for g in range(cfg.ngroups):
            blocks = list(range(g * G, min((g + 1) * G, nb)))
            gt = {0: xtiles.pop(g)}
            for h in range(1, NS):
                gt[h] = emit_gathers(g, h, gps[h - 1])
            if g + pref + 1 < cfg.ngroups:
                xtiles[g + pref + 1] = emit_gathers(g + pref + 1, 0, gp0)
            nmm = sum(len(meta["mm_list"][g][h]) for h in range(NS))
            if nmm > 0:
                ptile = pp.tile([128, nmm * 128], bf, tag="p")
                nc.sync.dma_start(
                    ptile[:], pmat_d[:, j0 * 128 : (j0 + nmm) * 128]
                )
            psums = {
                b: sp.tile([128, H2], f32, tag="ps", name=f"ps{psum_tag}_{b}")
                for b in blocks
            }
            started = dict.fromkeys(blocks, False)
            jj = 0
            for h in range(NS):
                for j, b in meta["mm_list"][g][h]:
                    nc.tensor.matmul(
                        psums[b][:],
                        lhsT=ptile[:, jj * 128 : (jj + 1) * 128],
                        rhs=gt[h][j // CALL_CHUNKS][:, j % CALL_CHUNKS, :],
                        start=not started[b],
                        stop=False,
                    )
                    started[b] = True
                    jj += 1
            for b in blocks:
                nc.tensor.matmul(
                    psums[b][:],
                    lhsT=ones_t[:1, :],
                    rhs=brow[:1, :],
                    start=not started[b],
                    stop=True,
                )
                out_cb(b, psums[b])
            j0 += nmm

    with tile.TileContext(nc) as tc:
        with ExitStack() as top:
            const = top.enter_context(tc.tile_pool(name="const", bufs=1))
            w1_s = const.tile([128, kin * cfg.h1], bf)
            nc.sync.dma_start(w1_s[:], w1_d[:, :])
            wg1_s = const.tile([128, k1 * H2], bf)
            nc.sync.dma_start(wg1_s[:], wg1_d[:, :])
            wg2_s = const.tile([128, k2 * H2], bf)
            nc.sync.dma_start(wg2_s[:], wg2_d[:, :])
            wl2_s = const.tile([128, k2 * OUT], bf)
            nc.sync.dma_start(wl2_s[:], wl2_d[:, :])
            b1_s = const.tile([128, kin], f32)
            nc.sync.dma_start(b1_s[:], b1_d[:, :])
            bg1_s = const.tile([1, H2], bf)
            nc.sync.dma_start(bg1_s[:], bg1_d[:, :])
            bg2_s = const.tile([1, H2], bf)
            nc.sync.dma_start(bg2_s[:], bg2_d[:, :])
            bl2_s = const.tile([1, OUT], bf)
            nc.sync.dma_start(bl2_s[:], bl2_d[:, :])
            idx_s = const.tile([128, meta["idxcols"]], i16)
            nc.sync.dma_start(idx_s[:], idx_d[:, :])
            ident = const.tile([128, 128], bf)
            make_identity(nc, ident[:])
            ones_t = const.tile([1, 128], bf)
            nc.gpsimd.memset(ones_t[:], 1.0)

            def sub_of_block(b):
                for s in range(NS):
                    if b < CUM[s + 1]:
                        return s
                raise AssertionError(b)

            def glocal_write(tileap, b, locs):
                s = sub_of_block(b)
                bb = b - CUM[s]
                nc.sync.dma_start(locs[s][bb * 128 : (bb + 1) * 128, :], tileap)

            # ---------------- L1: h1T[f, n] = sigmoid(W1.T @ X.T + b1)
            with ExitStack() as ph1:
                h1p = ph1.enter_context(tc.tile_pool(name="h1t", bufs=1))
                h1t = h1p.tile([128, k1 * npad], bf)
                with ExitStack() as px:
                    xp = px.enter_context(tc.tile_pool(name="xt", bufs=1))
                    psp = px.enter_context(
                        tc.tile_pool(name="ps1", bufs=4, space="PSUM")
                    )
                    # one tile per k-slice so the first matmuls start as
                    # soon as the first slice of X lands
                    xt_ks = []
                    for kt in range(kin):
                        xk = xp.tile([128, npad], bf, tag=f"x{kt}")
                        nc.sync.dma_start(
                            xk[:], xt_d[:, kt * npad : (kt + 1) * npad]
                        )
                        xt_ks.append(xk)
                    nsl = [
                        (i * 512, min((i + 1) * 512, npad))
                        for i in range(math.ceil(npad / 512))
                    ]
                    for f1t in range(k1):
                        for a, b_ in nsl:
                            nw = b_ - a
                            ps = psp.tile([128, 512], f32, tag="ps")
                            for kt in range(kin):
                                nc.tensor.matmul(
                                    ps[:, :nw],
                                    lhsT=w1_s[
                                        :,
                                        kt * cfg.h1
                                        + f1t * 128 : kt * cfg.h1
                                        + f1t * 128
                                        + 128,
                                    ],
                                    rhs=xt_ks[kt][:, a:b_],
                                    start=(kt == 0),
                                    stop=(kt == kin - 1),
                                )
                            nc.scalar.activation(
                                h1t[:, f1t * npad + a : f1t * npad + b_],
                                ps[:, :nw],
                                AF.Sigmoid,
                                bias=b1_s[:, f1t : f1t + 1],
                            )

                # ---------------- L2a: g1[n, h2] = h1 @ Wg1  (lhsT = h1T)
                with ExitStack() as p2:
                    psp2 = p2.enter_context(
                        tc.tile_pool(name="ps2", bufs=4, space="PSUM")
                    )
                    tp2 = p2.enter_context(tc.tile_pool(name="g1t", bufs=3))
                    for b in range(nb):
                        ps = psp2.tile([128, H2], f32, tag="ps")
                        for kt in range(k1):
                            nc.tensor.matmul(
                                ps[:],
                                lhsT=h1t[
                                    :, kt * npad + b * 128 : kt * npad + b * 128 + 128
                                ],
                                rhs=wg1_s[:, kt * H2 : (kt + 1) * H2],
                                start=(kt == 0),
                                stop=(kt == k1 - 1),
                            )
                        g1tile = tp2.tile([128, H2], bf, tag="g1")
                        nc.vector.tensor_copy(g1tile[:], ps[:])
                        glocal_write(g1tile[:], b, g1l)
                        s = sub_of_block(b)
                        if b == CUM[s + 1] - 1:
                            allgather(g1l[s], g1t[s])

            # ------- spmm1 -> h2, fused with L3a (g2 = relu(spmm1) @ Wg2)
            # per block: relu into h2r, transpose, matmul by Wg2, write the
            # g2 local shard; AllGathers for g2 fire as soon as their shard
            # is complete so they overlap spmm1's tail groups.
            with ExitStack() as ph2:
                h2p = ph2.enter_context(tc.tile_pool(name="h2res", bufs=1))
                h2r = h2p.tile([128, nb * H2], bf)

                with ExitStack() as ps1:
                    tps = ps1.enter_context(
                        tc.tile_pool(name="tps", bufs=2, space="PSUM")
                    )
                    psp3 = ps1.enter_context(
                        tc.tile_pool(name="ps3", bufs=2, space="PSUM")
                    )
                    tp3 = ps1.enter_context(tc.tile_pool(name="l3t", bufs=3))

                    def cb1(b, psum):
                        nc.scalar.activation(
                            h2r[:, b * H2 : (b + 1) * H2], psum[:], AF.Relu
                        )
                        h2T = tp3.tile([128, k2, 128], bf, tag="h2T")
                        for kt in range(k2):
                            pt = tps.tile([128, 128], bf, tag="pt")
                            nc.tensor.transpose(
                                pt[:],
                                h2r[
                                    :,
                                    b * H2 + kt * 128 : b * H2 + (kt + 1) * 128,
                                ],
                                ident[:],
                            )
                            nc.vector.tensor_copy(h2T[:, kt, :], pt[:])
                        ps = psp3.tile([128, H2], f32, tag="ps")
                        for kt in range(k2):
                            nc.tensor.matmul(
                                ps[:],
                                lhsT=h2T[:, kt, :],
                                rhs=wg2_s[:, kt * H2 : (kt + 1) * H2],
                                start=(kt == 0),
                                stop=(kt == k2 - 1),
                            )
                        g2tile = tp3.tile([128, H2], bf, tag="g2")
                        nc.vector.tensor_copy(g2tile[:], ps[:])
                        glocal_write(g2tile[:], b, g2l)
                        s = sub_of_block(b)
                        if b == CUM[s + 1] - 1 and s < NS - 1:
                            # last sub's AllGather is emitted by spmm2's
                            # pre_hook so its gathers aren't queued
                            # behind this instruction
                            allgather(g2l[s], g2t[s])

                    spmm(
                        tc,
                        ps1,
                        nc,
                        g1t,
                        idx_s,
                        ones_t,
                        bg1_s,
                        "a",
                        cb1,
                        pref=2,
                    )

            # ---------------- spmm2 + L4 fused per block
            with ExitStack() as ps2x:
                tps4 = ps2x.enter_context(
                    tc.tile_pool(name="tps4", bufs=2, space="PSUM")
                )
                psp4 = ps2x.enter_context(
                    tc.tile_pool(name="ps4", bufs=2, space="PSUM")
                )
                tp4 = ps2x.enter_context(tc.tile_pool(name="l4t", bufs=3))

                def cb2(b, psum):
                    h3t = tp4.tile([128, H2], bf, tag="h3")
                    nc.scalar.activation(h3t[:], psum[:], AF.Relu)
                    h3T = tp4.tile([128, k2, 128], bf, tag="h3T")
                    for kt in range(k2):
                        pt = tps4.tile([128, 128], bf, tag="pt")
                        nc.tensor.transpose(
                            pt[:], h3t[:, kt * 128 : (kt + 1) * 128], ident[:]
                        )
                        nc.vector.tensor_copy(h3T[:, kt, :], pt[:])
                    ps4 = psp4.tile([128, OUT], f32, tag="ps")
                    for kt in range(k2):
                        nc.tensor.matmul(
                            ps4[:],
                            lhsT=h3T[:, kt, :],
                            rhs=wl2_s[:, kt * OUT : (kt + 1) * OUT],
                            start=(kt == 0),
                            stop=False,
                        )
                    nc.tensor.matmul(
                        ps4[:],
                        lhsT=ones_t[:1, :],
                        rhs=bl2_s[:1, :],
                        start=False,
                        stop=True,
                    )
                    yt = tp4.tile([128, OUT], f32, tag="y")
                    nc.vector.tensor_copy(yt[:], ps4[:])
                    nc.sync.dma_start(y_d[b * 128 : (b + 1) * 128, :], yt[:])

                spmm(
                    tc,
                    ps2x,
                    nc,
                    g2t,
                    idx_s,
                    ones_t,
                    bg2_s,
                    "b",
                    cb2,
                    pref=3,
                    pre_hook=lambda: allgather(g2l[NS - 1], g2t[NS - 1]),
                )

    nc.compile()
    return nc


# ---------------------------------------------------------------- driver

_CACHE = {}


def run(inputs, cfg: Cfg = FULL, trace=False, tmpdir=None):
    meta, in_maps = prep_inputs(cfg, inputs)
    key = (cfg, meta["totch"], meta["idxcols"])
    if key not in _CACHE:
        _CACHE[key] = build(cfg, meta)
    nc = _CACHE[key]
    res = run_bass_kernel_spmd(
        nc,
        in_maps,
        core_ids=list(range(cfg.n_cores)),
        trace=trace,
        tmpdir=tmpdir,
    )
    npc = cfg.nodes_per_core
    out = np.empty((cfg.n_nodes, cfg.out_dim), np.float32)
    for c in range(cfg.n_cores):
        lo = c * npc
        hi = min((c + 1) * npc, cfg.n_nodes)
        out[lo:hi] = res.results[c]["y"][: hi - lo]
    return out, res


def kernel(**inputs) -> np.ndarray:
    out, _ = run(inputs, FULL, trace=False)
    return out


# revision 28
# speedup vs baseline: 1.1092x; 1.1092x over previous
"""GNN message-passing kernel for 8 Trainium2 NeuronCores (Bass/Tile).

Takes FULL inputs, shards nodes across 8 cores internally, runs the
4-layer GNN (dense -> spmm -> spmm -> dense) with bf16 AllGathers of the
hidden node table (split into X/Y sub-tables so the second collective
overlaps the first sub-table's gathers), and PE-matmul-based weighted
segment sums (host-built one-hot selector matrices), then gathers the
full output. dma_gather calls rotate over 4 SWDGE queues so descriptor
generation runs on all four Q7 core-pairs concurrently.
"""

import math
from contextlib import ExitStack
from dataclasses import dataclass

import ml_dtypes
import numpy as np

import concourse.bass as bass
import concourse.mybir as mybir
import concourse.tile as tile
from concourse import bacc
from concourse.bass_utils import run_bass_kernel_spmd
from concourse.masks import make_identity

BF16 = ml_dtypes.bfloat16
AF = mybir.ActivationFunctionType


@dataclass(frozen=True)
class Cfg:
    n_nodes: int = 50000
    n_edges: int = 800000
    in_dim: int = 512
    h1: int = 512
    h2: int = 256
    out_dim: int = 128
    n_cores: int = 8
    group_blocks: int = 3  # row-blocks per gather group
    sub_blocks: tuple = (25, 24)  # local row-blocks per sub-table

    @property
    def nodes_per_core(self):
        return math.ceil(self.n_nodes / self.n_cores)

    @property
    def npad(self):  # per-core padded nodes
        return math.ceil(self.nodes_per_core / 128) * 128

    @property
    def nblocks(self):
        return self.npad // 128

    @property
    def nsub(self):
        return len(self.sub_blocks)

    @property
    def sub_cumblk(self):  # cumulative block boundaries of the sub-tables
        c = [0]
        for nb_ in self.sub_blocks:
            c.append(c[-1] + nb_)
        return c

    @property
    def srows(self):  # per-core rows per sub-table
        return [nb_ * 128 for nb_ in self.sub_blocks]

    @property
    def stot(self):  # total rows per sub-table across cores
        return [r * self.n_cores for r in self.srows]

    @property
    def ngroups(self):
        return math.ceil(self.nblocks / self.group_blocks)


FULL = Cfg()

CALL_CHUNKS = 5  # chunks per dma_gather call (fits the SWDGE ring 3x over)


# ---------------------------------------------------------------- host prep


def edge_structure(cfg: Cfg, edge_row, edge_col, edge_weight):
    """Per-core edge streams with SPMD-uniform chunk counts.

    Edges are bucketed per (core, row-block, sub-table) where the
    sub-table (h=0: X, h=1: Y) is chosen by the column's local row so
    that each sub-table fits int16 gather indices.

    Returns (meta, per_core) where meta has the uniform chunk structure:
      meta['nch'][g][h]      total chunks in group g, sub-table h
      meta['chunk_blk'][g][h] list of block ids (one per chunk, ordered)
      meta['off16'][g][h]    idx-tile column offset (int16 cols)
      meta['totch']          total chunks
      meta['idxcols']        total int16 columns of the idx tensor
    per_core[c] = dict(idx=[128, idxcols] int16, pmat=[128, totch*128] bf16)
    """
    nc_, npad, nb, G = (cfg.n_cores, cfg.npad, cfg.nblocks, cfg.group_blocks)
    npc = cfg.nodes_per_core
    NS = cfg.nsub
    assert all(t <= 32767 for t in cfg.stot)
    bounds = np.array([b * 128 for b in cfg.sub_cumblk])  # local-row bounds
    srows = np.array(cfg.srows)

    core_of = edge_row // npc
    lr_all = edge_row - core_of * npc  # local row
    ccore = edge_col // npc
    clocal = edge_col - ccore * npc
    sub_all = np.searchsorted(bounds, clocal, side="right") - 1  # sub-table
    cl_all = ccore * srows[sub_all] + (clocal - bounds[sub_all])

    # bucket edges per (core, group, sub); blocks stay ordered inside a
    # group stream so a chunk spans at most a couple of adjacent blocks
    per = {}
    ngroups = cfg.ngroups
    counts = np.zeros((nc_, ngroups, NS), np.int64)
    for c in range(nc_):
        m = core_of == c
        lr, cl, hf, w = lr_all[m], cl_all[m], sub_all[m], edge_weight[m]
        blk = lr // 128
        grp = blk // G
        order = np.lexsort((lr, blk, hf, grp))
        per[c] = (lr[order], cl[order], hf[order], w[order], blk[order], grp[order])
        np.add.at(counts[c], (grp, hf), 1)

    # uniform chunks per (group, sub) = max over cores
    chunks_gh = np.ceil(counts / 128.0).astype(np.int64).max(axis=0)

    nch = [[int(chunks_gh[g, h]) for h in range(NS)] for g in range(ngroups)]
    off16 = [[0] * NS for _ in range(ngroups)]
    tot16 = 0
    for g in range(ngroups):
        for h in range(NS):
            off16[g][h] = tot16
            tot16 += nch[g][h] * 8  # 128 idx per chunk -> 8 int16 cols

    # per-core block layout inside each (g, h) stream, then the UNION of
    # (chunk, block) MM pairs across cores (a chunk near a per-core block
    # boundary may need MMs into two adjacent blocks)
    mm_sets = [[{} for _ in range(NS)] for _ in range(ngroups)]  # {(j, b)}
    core_spans = {}  # (c, g, h) -> list of (block, start_slot, end_slot)
    for c in range(nc_):
        lr, cl, hf, w, blk, grp = per[c]
        for g in range(ngroups):
            for h in range(NS):
                sel = (grp == g) & (hf == h)
                bsel = blk[sel]
                spans = []
                pos = 0
                for b in range(g * G, min((g + 1) * G, nb)):
                    nB = int((bsel == b).sum())
                    if nB:
                        spans.append((b, pos, pos + nB))
                        for j in range(pos // 128, (pos + nB - 1) // 128 + 1):
                            mm_sets[g][h].setdefault((j, b), True)
                    pos += nB
                core_spans[(c, g, h)] = spans

    mm_list = [
        [sorted(mm_sets[g][h].keys()) for h in range(NS)]
        for g in range(ngroups)
    ]
    totmm = sum(
        len(mm_list[g][h]) for g in range(ngroups) for h in range(NS)
    )
    totch = int(chunks_gh.sum())

    meta = dict(
        nch=nch,
        mm_list=mm_list,
        off16=off16,
        totch=totch,
        totmm=totmm,
        idxcols=max(tot16, 8),
    )

    # global pmat column index of each (g, h, j, b) MM
    mm_col = {}
    jj = 0
    for g in range(ngroups):
        for h in range(NS):
            for key in mm_list[g][h]:
                mm_col[(g, h) + key] = jj
                jj += 1

    per_core = []
    for c in range(nc_):
        lr, cl, hf, w, blk, grp = per[c]
        idx_flat = np.zeros(meta["idxcols"] * 16, np.int16)
        pmat = np.zeros((128, totmm * 128), BF16)
        for g in range(ngroups):
            for h in range(NS):
                sel = (grp == g) & (hf == h)
                e_cl, e_lr, e_w = cl[sel], lr[sel], w[sel]
                n = len(e_cl)
                if n == 0:
                    continue
                # idx stream (pad slots keep idx 0 -> harmless row-0 reads)
                base16 = off16[g][h]
                i_in = np.arange(n)
                idx_flat[(base16 + i_in // 16) * 16 + (i_in % 16)] = (
                    e_cl.astype(np.int16)
                )
                # P matrices per (chunk, block) MM
                for b, s0, s1 in core_spans[(c, g, h)]:
                    i_e = np.arange(s0, s1)
                    jloc = i_e // 128
                    slot = i_e % 128
                    col = np.array(
                        [mm_col[(g, h, int(j), b)] for j in jloc], np.int64
                    )
                    r = e_lr[s0:s1] - b * 128
                    pmat[slot, col * 128 + r] = e_w[s0:s1].astype(BF16)
        idx_mat = idx_flat.reshape(meta["idxcols"], 16).T  # [16, idxcols]
        idx_mat = np.tile(idx_mat, (8, 1))  # replicate to 128 partitions
        per_core.append(dict(idx=np.ascontiguousarray(idx_mat), pmat=pmat))

    return meta, per_core


def prep_inputs(cfg: Cfg, inputs):
    """Shard + lay out all per-core input tensors."""
    f = inputs["features"].astype(np.float32)
    meta, per_edge = edge_structure(
        cfg,
        inputs["edge_row"].astype(np.int64),
        inputs["edge_col"].astype(np.int64),
        inputs["edge_weight"].astype(np.float32),
    )
    kin = cfg.in_dim // 128
    k1 = cfg.h1 // 128
    k2 = cfg.h2 // 128

    def wlayout(w, kt):
        # [K, M] -> [128, kt*M] with [:, i*M:(i+1)*M] = w[i*128:(i+1)*128]
        K, M = w.shape
        return (
            w.reshape(kt, 128, M).transpose(1, 0, 2).reshape(128, kt * M)
        ).astype(BF16)

    w1 = wlayout(inputs["W_lin1"].astype(np.float32), kin)
    wg1 = wlayout(inputs["W_g1"].astype(np.float32), k1)
    wg2 = wlayout(inputs["W_g2"].astype(np.float32), k2)
    wl2 = wlayout(inputs["W_lin2"].astype(np.float32), k2)
    b1 = (
        inputs["b_lin1"].astype(np.float32).reshape(kin, 128).T.copy()
    )  # [128, kin]
    bg1 = inputs["b_g1"].astype(BF16).reshape(1, cfg.h2)
    bg2 = inputs["b_g2"].astype(BF16).reshape(1, cfg.h2)
    bl2 = inputs["b_lin2"].astype(BF16).reshape(1, cfg.out_dim)

    npc, npad = cfg.nodes_per_core, cfg.npad
    in_maps = []
    for c in range(cfg.n_cores):
        lo = c * npc
        hi = min((c + 1) * npc, cfg.n_nodes)
        xc = np.zeros((npad, cfg.in_dim), np.float32)
        xc[: hi - lo] = f[lo:hi]
        # XT layout [128, kin*npad]: [:, i*npad:(i+1)*npad] = x.T[i*128:...]
        xt = (
            xc.T.reshape(kin, 128, npad)
            .transpose(1, 0, 2)
            .reshape(128, kin * npad)
        ).astype(BF16)
        in_maps.append(
            {
                "xt": np.ascontiguousarray(xt),
                "w1": w1,
                "wg1": wg1,
                "wg2": wg2,
                "wl2": wl2,
                "b1": b1,
                "bg1": bg1,
                "bg2": bg2,
                "bl2": bl2,
                "idx": per_edge[c]["idx"],
                "pmat": per_edge[c]["pmat"],
            }
        )
    return meta, in_maps


# ---------------------------------------------------------------- kernel IR


def build(cfg: Cfg, meta):
    nc = bacc.Bacc(
        "TRN2",
        target_bir_lowering=False,
        debug=False,
        num_devices=cfg.n_cores,
        num_swdge_queues=4,
        dynamic_dma_scratch_size=32768,
    )
    bf = mybir.dt.bfloat16
    f32 = mybir.dt.float32
    i16 = mybir.dt.int16
    kin = cfg.in_dim // 128
    k1 = cfg.h1 // 128
    k2 = cfg.h2 // 128
    npad, nb, G, H2, OUT = (
        cfg.npad,
        cfg.nblocks,
        cfg.group_blocks,
        cfg.h2,
        cfg.out_dim,
    )
    NS = cfg.nsub
    CUM = cfg.sub_cumblk
    totch = meta["totch"]

    xt_d = nc.dram_tensor("xt", [128, kin * npad], bf, kind="ExternalInput").ap()
    w1_d = nc.dram_tensor("w1", [128, kin * cfg.h1], bf, kind="ExternalInput").ap()
    wg1_d = nc.dram_tensor("wg1", [128, k1 * H2], bf, kind="ExternalInput").ap()
    wg2_d = nc.dram_tensor("wg2", [128, k2 * H2], bf, kind="ExternalInput").ap()
    wl2_d = nc.dram_tensor("wl2", [128, k2 * OUT], bf, kind="ExternalInput").ap()
    b1_d = nc.dram_tensor("b1", [128, kin], f32, kind="ExternalInput").ap()
    bg1_d = nc.dram_tensor("bg1", [1, H2], bf, kind="ExternalInput").ap()
    bg2_d = nc.dram_tensor("bg2", [1, H2], bf, kind="ExternalInput").ap()
    bl2_d = nc.dram_tensor("bl2", [1, OUT], bf, kind="ExternalInput").ap()
    idx_d = nc.dram_tensor(
        "idx", [128, meta["idxcols"]], i16, kind="ExternalInput"
    ).ap()
    pmat_d = nc.dram_tensor(
        "pmat", [128, meta["totmm"] * 128], bf, kind="ExternalInput"
    ).ap()
    y_d = nc.dram_tensor("y", [npad, OUT], f32, kind="ExternalOutput").ap()

    g1l = [
        nc.dram_tensor(f"g1l{s}", [cfg.srows[s], H2], bf).ap()
        for s in range(NS)
    ]
    g2l = [
        nc.dram_tensor(f"g2l{s}", [cfg.srows[s], H2], bf).ap()
        for s in range(NS)
    ]
    g1t = [
        nc.dram_tensor(f"g1t{s}", [cfg.stot[s], H2], bf, addr_space="Shared").ap()
        for s in range(NS)
    ]
    g2t = [
        nc.dram_tensor(f"g2t{s}", [cfg.stot[s], H2], bf, addr_space="Shared").ap()
        for s in range(NS)
    ]

    rg = [list(range(cfg.n_cores))]

    def allgather(local, table):
        nc.gpsimd.collective_compute(
            "AllGather",
            mybir.AluOpType.bypass,
            replica_groups=rg,
            ins=[local[:, :]],
            outs=[table[:, :]],
        )

    def spmm(
        tc,
        ctx,
        nc,
        tables,
        idx_s,
        ones_t,
        brow,
        psum_tag,
        out_cb,
        pref,
        pre_hook=None,
    ):
        """Weighted segment-sum of gathered table rows, per row-block.

        Sub-table-0 gathers are emitted `pref` groups ahead of their
        consuming matmuls so the GpSimd queue never head-of-line blocks
        on the later sub-tables' AllGathers. `pre_hook` (e.g. the last
        sub-table's AllGather) is emitted after the prefetch gathers so
        they aren't queued behind it.
        """
        calls_per_gh = max(
            math.ceil(meta["nch"][g][h] / CALL_CHUNKS)
            for g in range(cfg.ngroups)
            for h in range(NS)
        )
        gp0 = ctx.enter_context(
            tc.tile_pool(name=f"g0{psum_tag}", bufs=(pref + 2) * calls_per_gh)
        )
        gps = [
            ctx.enter_context(
                tc.tile_pool(name=f"g{h}{psum_tag}", bufs=2 * calls_per_gh)
            )
            for h in range(1, NS)
        ]
        pp = ctx.enter_context(tc.tile_pool(name=f"pm{psum_tag}", bufs=2))
        sp = ctx.enter_context(
            tc.tile_pool(name=f"ps{psum_tag}", bufs=G + 1, space="PSUM")
        )
        qctr = [0]

        def emit_gathers(g, h, pool):
            n = meta["nch"][g][h]
            tiles = []
            for lo in range(0, n, CALL_CHUNKS):
                ns = min(CALL_CHUNKS, n - lo)
                t = pool.tile([128, CALL_CHUNKS, H2], bf, tag="g")
                o16 = meta["off16"][g][h] + lo * 8
                nc.gpsimd.dma_gather(
                    out_ap=t[:, :ns, :],
                    in_ap=tables[h][:, :],
                    idxs_ap=idx_s[:, o16 : o16 + ns * 8],
                    num_idxs=ns * 128,
                    num_idxs_reg=ns * 128,
                    elem_size=H2,
                    single_packet=False,
                    queue_num=qctr[0] % 4,
                )
                qctr[0] += 1
                tiles.append(t)
            return tiles

        xtiles = {}
        for gg in range(min(pref + 1, cfg.ngroups)):
            xtiles[gg] = emit_gathers(gg, 0, gp0)
        if pre_hook is not None:
            pre_hook()

        j0 = 0
        for g in range(cfg.ngroups):
            blocks = list(range(g * G, min((g + 1) * G, nb)))
            gt = {0: xtiles.pop(g)}
            for h in range(1, NS):
                gt[h] = emit_gathers(g, h, gps[h - 1])
            if g + pref + 1 < cfg.ngroups:
                xtiles[g + pref + 1] = emit_gathers(g + pref + 1, 0, gp0)
            nmm = sum(len(meta["mm_list"][g][h]) for h in range(NS))
            if nmm > 0:
                ptile = pp.tile([128, nmm * 128], bf, tag="p")
                nc.sync.dma_start(
                    ptile[:], pmat_d[:, j0 * 128 : (j0 + nmm) * 128]
                )
            psums = {
                b: sp.tile([128, H2], f32, tag="ps", name=f"ps{psum_tag}_{b}")
                for b in blocks
            }
            started = dict.fromkeys(blocks, False)
            jj = 0
            for h in range(NS):
                for j, b in meta["mm_list"][g][h]:
                    nc.tensor.matmul(
                        psums[b][:],
                        lhsT=ptile[:, jj * 128 : (jj + 1) * 128],
                        rhs=gt[h][j // CALL_CHUNKS][:, j % CALL_CHUNKS, :],
                        start=not started[b],
                        stop=False,
                    )
                    started[b] = True
                    jj += 1
            for b in blocks:
                nc.tensor.matmul(
                    psums[b][:],
                    lhsT=ones_t[:1, :],
                    rhs=brow[:1, :],
                    start=not started[b],
                    stop=True,
                )
                out_cb(b, psums[b])
            j0 += nmm

    with tile.TileContext(nc) as tc:
        with ExitStack() as top:
            const = top.enter_context(tc.tile_pool(name="const", bufs=1))
            w1_s = const.tile([128, kin * cfg.h1], bf)
            nc.sync.dma_start(w1_s[:], w1_d[:, :])
            wg1_s = const.tile([128, k1 * H2], bf)
            nc.sync.dma_start(wg1_s[:], wg1_d[:, :])
            wg2_s = const.tile([128, k2 * H2], bf)
            nc.sync.dma_start(wg2_s[:], wg2_d[:, :])
            wl2_s = const.tile([128, k2 * OUT], bf)
            nc.sync.dma_start(wl2_s[:], wl2_d[:, :])
            b1_s = const.tile([128, kin], f32)
            nc.sync.dma_start(b1_s[:], b1_d[:, :])
            bg1_s = const.tile([1, H2], bf)
            nc.sync.dma_start(bg1_s[:], bg1_d[:, :])
            bg2_s = const.tile([1, H2], bf)
            nc.sync.dma_start(bg2_s[:], bg2_d[:, :])
            bl2_s = const.tile([1, OUT], bf)
            nc.sync.dma_start(bl2_s[:], bl2_d[:, :])
            idx_s = const.tile([128, meta["idxcols"]], i16)
            nc.sync.dma_start(idx_s[:], idx_d[:, :])
            ident = const.tile([128, 128], bf)
            make_identity(nc, ident[:])
            ones_t = const.tile([1, 128], bf)
            nc.gpsimd.memset(ones_t[:], 1.0)

            def sub_of_block(b):
                for s in range(NS):
                    if b < CUM[s + 1]:
                        return s
                raise AssertionError(b)

            def glocal_write(tileap, b, locs):
                s = sub_of_block(b)
                bb = b - CUM[s]
                nc.sync.dma_start(locs[s][bb * 128 : (bb + 1) * 128, :], tileap)

            # ---------------- L1: h1T[f, n] = sigmoid(W1.T @ X.T + b1)
            with ExitStack() as ph1:
                h1p = ph1.enter_context(tc.tile_pool(name="h1t", bufs=1))
                h1t = h1p.tile([128, k1 * npad], bf)
                with ExitStack() as px:
                    xp = px.enter_context(tc.tile_pool(name="xt", bufs=1))
                    psp = px.enter_context(
                        tc.tile_pool(name="ps1", bufs=4, space="PSUM")
                    )
                    # one tile per k-slice so the first matmuls start as
                    # soon as the first slice of X lands
                    xt_ks = []
                    for kt in range(kin):
                        xk = xp.tile([128, npad], bf, tag=f"x{kt}")
                        nc.sync.dma_start(
                            xk[:], xt_d[:, kt * npad : (kt + 1) * npad]
                        )
                        xt_ks.append(xk)
                    nsl = [
                        (i * 512, min((i + 1) * 512, npad))
                        for i in range(math.ceil(npad / 512))
                    ]
                    for f1t in range(k1):
                        for a, b_ in nsl:
                            nw = b_ - a
                            ps = psp.tile([128, 512], f32, tag="ps")
                            for kt in range(kin):
                                nc.tensor.matmul(
                                    ps[:, :nw],
                                    lhsT=w1_s[
                                        :,
                                        kt * cfg.h1
                                        + f1t * 128 : kt * cfg.h1
                                        + f1t * 128
                                        + 128,
                                    ],
                                    rhs=xt_ks[kt][:, a:b_],
                                    start=(kt == 0),
                                    stop=(kt == kin - 1),
                                )
                            nc.scalar.activation(
                                h1t[:, f1t * npad + a : f1t * npad + b_],
                                ps[:, :nw],
                                AF.Sigmoid,
                                bias=b1_s[:, f1t : f1t + 1],
                            )

                # ---------------- L2a: g1[n, h2] = h1 @ Wg1  (lhsT = h1T)
                with ExitStack() as p2:
                    psp2 = p2.enter_context(
                        tc.tile_pool(name="ps2", bufs=4, space="PSUM")
                    )
                    tp2 = p2.enter_context(tc.tile_pool(name="g1t", bufs=3))
                    for b in range(nb):
                        ps = psp2.tile([128, H2], f32, tag="ps")
                        for kt in range(k1):
                            nc.tensor.matmul(
                                ps[:],
                                lhsT=h1t[
                                    :, kt * npad + b * 128 : kt * npad + b * 128 + 128
                                ],
                                rhs=wg1_s[:, kt * H2 : (kt + 1) * H2],
                                start=(kt == 0),
                                stop=(kt == k1 - 1),
                            )
                        g1tile = tp2.tile([128, H2], bf, tag="g1")
                        nc.vector.tensor_copy(g1tile[:], ps[:])
                        glocal_write(g1tile[:], b, g1l)
                        s = sub_of_block(b)
                        if b == CUM[s + 1] - 1:
                            allgather(g1l[s], g1t[s])

            # ------- spmm1 -> h2, fused with L3a (g2 = relu(spmm1) @ Wg2)
            # per block: relu into h2r, transpose, matmul by Wg2, write the
            # g2 local shard; AllGathers for g2 fire as soon as their shard
            # is complete so they overlap spmm1's tail groups.
            with ExitStack() as ph2:
                h2p = ph2.enter_context(tc.tile_pool(name="h2res", bufs=1))
                h2r = h2p.tile([128, nb * H2], bf)

                with ExitStack() as ps1:
                    tps = ps1.enter_context(
                        tc.tile_pool(name="tps", bufs=2, space="PSUM")
                    )
                    psp3 = ps1.enter_context(
                        tc.tile_pool(name="ps3", bufs=2, space="PSUM")
                    )
                    tp3 = ps1.enter_context(tc.tile_pool(name="l3t", bufs=3))

                    def cb1(b, psum):
                        nc.scalar.activation(
                            h2r[:, b * H2 : (b + 1) * H2], psum[:], AF.Relu
                        )
                        h2T = tp3.tile([128, k2, 128], bf, tag="h2T")
                        for kt in range(k2):
                            pt = tps.tile([128, 128], bf, tag="pt")
                            nc.tensor.transpose(
                                pt[:],
                                h2r[
                                    :,
                                    b * H2 + kt * 128 : b * H2 + (kt + 1) * 128,
                                ],
                                ident[:],
                            )
                            nc.vector.tensor_copy(h2T[:, kt, :], pt[:])
                        ps = psp3.tile([128, H2], f32, tag="ps")
                        for kt in range(k2):
                            nc.tensor.matmul(
                                ps[:],
                                lhsT=h2T[:, kt, :],
                                rhs=wg2_s[:, kt * H2 : (kt + 1) * H2],
                                start=(kt == 0),
                                stop=(kt == k2 - 1),
                            )
                        g2tile = tp3.tile([128, H2], bf, tag="g2")
                        nc.vector.tensor_copy(g2tile[:], ps[:])
                        glocal_write(g2tile[:], b, g2l)
                        s = sub_of_block(b)
                        if b == CUM[s + 1] - 1 and s < NS - 1:
                            # last sub's AllGather is emitted by spmm2's
                            # pre_hook so its gathers aren't queued
                            # behind this instruction
                            allgather(g2l[s], g2t[s])

                    spmm(
                        tc,
                        ps1,
                        nc,
                        g1t,
                        idx_s,
                        ones_t,
                        bg1_s,
                        "a",
                        cb1,
                        pref=2,
                    )

            # ---------------- spmm2 + L4 fused per block
            with ExitStack() as ps2x:
                tps4 = ps2x.enter_context(
                    tc.tile_pool(name="tps4", bufs=2, space="PSUM")
                )
                psp4 = ps2x.enter_context(
                    tc.tile_pool(name="ps4", bufs=2, space="PSUM")
                )
                tp4 = ps2x.enter_context(tc.tile_pool(name="l4t", bufs=3))

                def cb2(b, psum):
                    h3t = tp4.tile([128, H2], bf, tag="h3")
                    nc.scalar.activation(h3t[:], psum[:], AF.Relu)
                    h3T = tp4.tile([128, k2, 128], bf, tag="h3T")
                    for kt in range(k2):
                        pt = tps4.tile([128, 128], bf, tag="pt")
                        nc.tensor.transpose(
                            pt[:], h3t[:, kt * 128 : (kt + 1) * 128], ident[:]
                        )
                        nc.vector.tensor_copy(h3T[:, kt, :], pt[:])
                    ps4 = psp4.tile([128, OUT], f32, tag="ps")
                    for kt in range(k2):
                        nc.tensor.matmul(
                            ps4[:],
                            lhsT=h3T[:, kt, :],
                            rhs=wl2_s[:, kt * OUT : (kt + 1) * OUT],
                            start=(kt == 0),
                            stop=False,
                        )
                    nc.tensor.matmul(
                        ps4[:],
                        lhsT=ones_t[:1, :],
                        rhs=bl2_s[:1, :],
                        start=False,
                        stop=True,
                    )
                    yt = tp4.tile([128, OUT], f32, tag="y")
                    nc.vector.tensor_copy(yt[:], ps4[:])
                    nc.sync.dma_start(y_d[b * 128 : (b + 1) * 128, :], yt[:])

                spmm(
                    tc,
                    ps2x,
                    nc,
                    g2t,
                    idx_s,
                    ones_t,
                    bg2_s,
                    "b",
                    cb2,
                    pref=3,
                    pre_hook=lambda: allgather(g2l[NS - 1], g2t[NS - 1]),
                )

    nc.compile()
    return nc


# ---------------------------------------------------------------- driver

_CACHE = {}


def run(inputs, cfg: Cfg = FULL, trace=False, tmpdir=None):
    meta, in_maps = prep_inputs(cfg, inputs)
    key = (cfg, meta["totch"], meta["idxcols"])
    if key not in _CACHE:
        _CACHE[key] = build(cfg, meta)
    nc = _CACHE[key]
    res = run_bass_kernel_spmd(
        nc,
        in_maps,
        core_ids=list(range(cfg.n_cores)),
        trace=trace,
        tmpdir=tmpdir,
    )
    npc = cfg.nodes_per_core
    out = np.empty((cfg.n_nodes, cfg.out_dim), np.float32)
    for c in range(cfg.n_cores):
        lo = c * npc
        hi = min((c + 1) * npc, cfg.n_nodes)
        out[lo:hi] = res.results[c]["y"][: hi - lo]
    return out, res


def kernel(**inputs) -> np.ndarray:
    out, _ = run(inputs, FULL, trace=False)
    return out
